# revision 16
# baseline (speedup 1.0000x reference)
"""Trainium2 Bass kernel for the 2-stack GRU decoder with 5-wide sliding
window attention (nn_DEC_59880434041064).

Strategy: pure data parallel over batch (1024 -> 8 cores x 128).
Per-core layout keeps features on partitions and batch on the free dim so
the sequential GRU needs no transposes.  The attention + output projection
is algebraically collapsed on the host: only four per-(b,t) scalar fields
(window score s_p, q = u_c.r, p = u_r.r, g = w_o.r) are needed, computed by
an N=4 matmul against each fresh hidden state, so no [H,B,T] context GEMM
ever runs on device.

v3 perf rework (on top of the fp16 v2), 2317us -> 1867us:
 - every per-step tile is PER-LAYER (incl. the z-gate psum + sigmoid that
   v2 shared): the tile tracker works at whole-tile granularity, so any
   shared tile serializes one layer's recurrence chain behind the other.
 - the (z-1)*h blend term moves to the idle Pool/GPSIMD engine as two ops
   (tensor_scalar z-1, then mult by h; Pool's ISA has no STT), taking
   ~650ns/step off the DVE, the busiest engine.
 - matmuls are emitted x-side/bias-indicator first (they open each psum
   bank with its single start=True), recurrent hh matmuls last, so the
   in-order PE queue never parks on ready work.  NOTE start=True clears
   has_written for the whole 2KB psum zero region, hence exactly one per
   bank; later matmuls rely on per-element has_written.
 - score matmuls lag one step so their fin1 wait cannot block the next
   step's hh matmuls at the PE queue head; x chunks are prefetched.
 - score fields stay SBUF-resident (per-chunk ACT copy into a persistent
   staging tile); the old SBUF->DRAM->SBUF round trip cost ~240us of
   tail latency.
 - bigger SBUF pool rotations let the tile scheduler elide WAR waits.
"""

import os

import numpy as np

import concourse.bass as bass
import concourse.mybir as mybir
import concourse.tile as tile
from concourse import bacc
from concourse.bass_utils import run_bass_kernel_spmd

FP = mybir.dt.float32
F16 = mybir.dt.float16
AL = mybir.AluOpType
AF = mybir.ActivationFunctionType

B, L, H, NIN = 1024, 512, 128, 2
L = int(os.environ.get("BASS_GRU_L", L))  # debug-size override
NCORES = 8
BL = B // NCORES            # 128 samples per core
ATTN, DLY = 5, 10
WIN = L - ATTN              # 507 valid attention positions (t >= 5)
XCH = min(64, L)            # timesteps per x-chunk DMA
SCH = min(64, L)            # timesteps per score-psum chunk

# fp16 weight blob column offsets (blob is [128, NW16] fp16)
HH_OFF = 0                  # 4 cells x 384
IH1_OFF = HH_OFF + 4 * 384  # 2 streams x 384 (layer-1 input weights)
IH0_OFF = IH1_OFF + 2 * 384  # 2 streams x 384 (layer-0 aug weights, rows 0..2)
SC_OFF = IH0_OFF + 2 * 384  # score vectors: 2 streams x 4 cols
BR_OFF = SC_OFF + 8         # bias rows for indicator MMs (see _prep_weights)
IND_OFF = BR_OFF + 5 * 128  # indicator rhs IND4 [4, 512]
NW16 = IND_OFF + 512

# fp32 aux blob (tiny): per-stream additive consts for the post phase
CONST_OFF = 0
NAUX = 2

_BUILT = {}


def _cell(s, l):
    return s * 2 + l


def _emit(tc, recv, wb16, wb32, onesd, out_ap, stag_out=None):
    nc = tc.nc
    import contextlib

    outer = contextlib.ExitStack()
    const = outer.enter_context(tc.tile_pool(name="const", bufs=1))
    wsb = const.tile([128, NW16], F16)
    nc.sync.dma_start(wsb[:], wb16[:])
    aux = const.tile([128, NAUX], FP)
    nc.sync.dma_start(aux[:], wb32[:])
    # persistent score staging: [b, t*8] fp16, written once per SCH chunk
    stag = const.tile([128, L * 8], FP)

    with contextlib.ExitStack() as ctx:
        xpool = ctx.enter_context(tc.tile_pool(name="xch", bufs=2))
        hpool = ctx.enter_context(tc.tile_pool(name="h", bufs=8))
        pw = ctx.enter_context(tc.tile_pool(name="pw", bufs=8))
        pg = [
            ctx.enter_context(tc.tile_pool(name=f"pg{l}", bufs=1, space="PSUM"))
            for l in range(2)
        ]
        psc = ctx.enter_context(tc.tile_pool(name="psc", bufs=2, space="PSUM"))

        def whh(s, l):  # [128, 384] lhsT
            o = HH_OFF + _cell(s, l) * 384
            return wsb[:, o:o + 384]

        def wih1(s):
            o = IH1_OFF + s * 384
            return wsb[:, o:o + 384]

        def wih0(s):  # [3, 384] augmented lhsT
            o = IH0_OFF + s * 384
            return wsb[0:3, o:o + 384]

        def scw(s):  # [128, 4]
            o = SC_OFF + s * 4
            return wsb[:, o:o + 4]

        # bias rows (lhsT for indicator matmuls); [2,128] per-stream groups:
        # slot 0 r-l1, 1 z-l1 (negated), 2 hn-l0, 3 hn-l1, 4 in-l1
        def brow(slot, k):
            o = BR_OFF + slot * 128
            return wsb[0:k, o:o + 128]

        def ind(k, n):  # [k, n] block indicator rhs
            return wsb[0:k, IND_OFF:IND_OFF + n]

        # initial hidden state (per layer, both streams concatenated)
        h = []
        for l in range(2):
            t0 = hpool.tile([128, 256], F16, tag=f"h{l}")
            nc.vector.memset(t0[:], 0.0)
            h.append(t0)

        def gru_stages(l, h_in, mm_rhs):
            """Build one fused (both-streams) GRU step for layer l as a dict
            of emit-stage callbacks, so the caller can interleave the two
            layers' independent chains stage-by-stage.  Every tile here is
            per-layer: sharing a tile across layers makes the tile tracker
            serialize one layer's chain behind the other (whole-tile
            granularity).

            mm_rhs: per-stream extra-input rhs ([3,128] aug x for l0,
            [128,128] r_l0 slice for l1).  Returns (stages, h')."""
            rp = pg[l].tile([128, 256], FP, tag=f"rp{l}", name=f"rp{l}")
            zp = pg[l].tile([128, 256], FP, tag=f"zp{l}", name=f"zp{l}")
            np_ = pg[l].tile([128, 512], FP, tag=f"np{l}", name=f"np{l}")
            nh = np_[:, 0:256]
            ni = np_[:, 256:512]
            rsb = pw.tile([128, 256], F16, tag=f"rsb{l}", name=f"rsb{l}")
            zsb = pw.tile([128, 256], F16, tag=f"zsb{l}", name=f"zsb{l}")
            tt = pw.tile([128, 256], F16, tag=f"tt{l}", name=f"tt{l}")
            uu = pw.tile([128, 256], F16, tag=f"uu{l}", name=f"uu{l}")
            nn_ = pw.tile([128, 256], F16, tag=f"nn{l}", name=f"nn{l}")
            zm = pw.tile([128, 256], F16, tag=f"zm{l}", name=f"zm{l}")
            hm = pw.tile([128, 256], F16, tag=f"hm{l}", name=f"hm{l}")
            zn = pw.tile([128, 256], F16, tag=f"zn{l}", name=f"zn{l}")
            hn_t = hpool.tile([128, 256], F16, tag=f"h{l}", name=f"hn{l}")

            def mm_early():
                # everything independent of this layer's fresh h: bias
                # indicators + x-side (l0) / lagged-rl0 (l1) input matmuls.
                # CRITICAL: start=True clears has_written for the whole 2KB
                # psum zero region (= bank), so each bank gets exactly ONE
                # start=True matmul, executed first; later matmuls rely on
                # per-element has_written (virgin columns are overwritten,
                # already-written ones accumulate).
                if l == 0:
                    nc.tensor.matmul(nh[:, 0:256], brow(2, 2), ind(2, 256),
                                     start=True, stop=False)
                    for s in range(2):
                        xr = mm_rhs[s]
                        wih = wih0(s)
                        c = s * 128
                        nc.tensor.matmul(rp[:, c:c + 128], wih[:, 0:128], xr,
                                         start=(s == 0), stop=False)
                        nc.tensor.matmul(zp[:, c:c + 128], wih[:, 128:256],
                                         xr, start=(s == 0), stop=False)
                        nc.tensor.matmul(ni[:, c:c + 128], wih[:, 256:384],
                                         xr, start=False, stop=False)
                else:
                    nc.tensor.matmul(rp[:, 0:256], brow(0, 2), ind(2, 256),
                                     start=True, stop=False)
                    nc.tensor.matmul(zp[:, 0:256], brow(1, 2), ind(2, 256),
                                     start=True, stop=False)
                    nc.tensor.matmul(nh[:, 0:256], brow(3, 2),
                                     ind(2, 256), start=True, stop=False)
                    nc.tensor.matmul(ni[:, 0:256], brow(4, 2),
                                     ind(2, 256), start=False, stop=False)
                    for s in range(2):
                        xr = mm_rhs[s]
                        wih = wih1(s)
                        c = s * 128
                        nc.tensor.matmul(rp[:, c:c + 128], wih[:, 0:128], xr,
                                         start=False, stop=False)
                        nc.tensor.matmul(zp[:, c:c + 128], wih[:, 128:256],
                                         xr, start=False, stop=False)
                        nc.tensor.matmul(ni[:, c:c + 128], wih[:, 256:384],
                                         xr, start=False, stop=False)

            def mm_hh(goff, dst):
                for s in range(2):
                    hs = h_in[:, s * 128:(s + 1) * 128]
                    c = s * 128
                    nc.tensor.matmul(dst[:, c:c + 128],
                                     whh(s, l)[:, goff:goff + 128], hs,
                                     start=False, stop=(s == 1))

            stages = {
                "early": mm_early,
                "hh_r": lambda: mm_hh(0, rp),
                "hh_z": lambda: mm_hh(128, zp),
                "hh_n": lambda: mm_hh(256, nh),
                "sr": lambda: nc.scalar.activation(rsb[:], rp[:], AF.Sigmoid),
                "sz": lambda: nc.scalar.activation(zsb[:], zp[:], AF.Sigmoid),
                "tt": lambda: nc.vector.tensor_mul(tt[:], nh[:], rsb[:]),
                "uu": lambda: nc.vector.tensor_add(uu[:], ni[:], tt[:]),
                "tanh": lambda: nc.scalar.activation(nn_[:], uu[:], AF.Tanh),
                # hm = (z'-1)*h on the otherwise-idle Pool engine; its
                # ISA has no scalar_tensor_tensor, so two legal ops
                "zm": lambda: nc.gpsimd.tensor_scalar_add(zm[:], zsb[:], -1.0),
                "hm": lambda: nc.gpsimd.tensor_mul(hm[:], zm[:], h_in[:]),
                "zn": lambda: nc.vector.tensor_mul(zn[:], zsb[:], nn_[:]),
                "fin": lambda: nc.vector.tensor_sub(hn_t[:], zn[:], hm[:]),
            }
            return stages, hn_t

        scp = None

        def emit_scores(h1_t, u):
            nonlocal scp
            if u % SCH == 0:
                scp = psc.tile([128, SCH * 8], FP, tag="scp", name=f"scp{u}")
            so = (u % SCH) * 8
            for s in range(2):
                nc.tensor.matmul(scp[:, so + s * 4:so + s * 4 + 4],
                                 h1_t[:, s * 128:(s + 1) * 128], scw(s),
                                 start=True, stop=True)

        def emit_stag_copy(u):
            # park the finished chunk's fields in the staging tile
            # (ACT copy: reads PSUM cheaply, DVE stays free)
            c0 = (u + 1 - SCH) * 8
            nc.scalar.copy(stag[:, c0:c0 + SCH * 8], scp[:])

        def load_chunk(c):
            xt = xpool.tile([3, XCH * 128], F16, tag="xch", name=f"xch{c}")
            src = recv[:, c * XCH:(c + 1) * XCH, :]
            dst = xt[0:2, :].rearrange("i (t b) -> i t b", b=128)
            nc.sync.dma_start(dst, src)
            nc.sync.dma_start(xt[2:3, :], onesd[:, 0:XCH * 128])
            return xt

        # wavefront: iteration tau emits l0(tau) and, LAG steps behind, the
        # l1 pair for tau-LAG -- the skew keeps the two recurrence chains
        # decoupled in every engine's in-order queue.  Scores for h1(u) are
        # emitted one iteration late so their fin1 wait never parks the PE
        # queue ahead of the next step's hh matmuls.
        LAG = 2
        h0_hist = [None] * LAG  # h0_hist[k] = h0 output of step tau-1-k
        xcur = load_chunk(0)
        xnext = None
        pend = None  # (h1 tile, u) for the delayed score matmuls
        for tau in range(L + LAG):
            st0 = st1 = None
            if tau < L:
                t = tau
                if t % XCH == 0 and t > 0:
                    xcur = xnext
                toff = t % XCH
                x_t = xcur[:, toff * 128:(toff + 1) * 128]
            if tau < L:
                st0, h0_new = gru_stages(0, h[0], [x_t, x_t])
            if tau >= LAG:
                u = tau - LAG
                # h0_hist stops rotating once tau >= L; account for that in
                # the drain iterations so the last l1 steps read fresh h0
                h0_u = h0_hist[LAG - 1 - max(0, tau - L)]
                rl0 = [h0_u[:, 0:128], h0_u[:, 128:256]]
                st1, h1_new = gru_stages(1, h[1], rl0)

            def run(st, k):
                if st is not None:
                    st[k]()

            # PE: no-fresh-h work first, then hh groups in dependency-
            # resolution order (fin0 lands before fin1); the delayed score
            # matmuls slot after hh_r0.
            run(st0, "early")
            run(st1, "early")
            run(st0, "hh_r")
            if pend is not None:
                emit_scores(*pend)
            run(st1, "hh_r")
            run(st0, "hh_n")
            run(st1, "hh_n")
            run(st0, "hh_z")
            run(st1, "hh_z")
            # ACT: each layer's z sigmoid right after its r sigmoid so
            # the blend helper (hm) unblocks early
            run(st0, "sr")
            run(st0, "sz")
            run(st1, "sr")
            run(st1, "sz")
            # Pool: blend helpers, each unblocked by its own z sigmoid
            run(st0, "zm")
            run(st0, "hm")
            run(st1, "zm")
            run(st1, "hm")
            # DVE: n-path pairs back-to-back, l0 first
            run(st0, "tt")
            run(st0, "uu")
            run(st1, "tt")
            run(st1, "uu")
            run(st0, "tanh")
            run(st1, "tanh")
            run(st0, "zn")
            run(st0, "fin")
            run(st1, "zn")
            run(st1, "fin")
            if pend is not None and (pend[1] + 1) % SCH == 0:
                emit_stag_copy(pend[1])
            pend = None
            # prefetch the next x chunk well before it is needed
            if tau < L and tau % XCH == 8 and t + XCH < L:
                xnext = load_chunk(t // XCH + 1)
            if tau >= LAG:
                pend = (h1_new, tau - LAG)
                h[1] = h1_new
            if tau < L:
                h0_hist = [h0_new] + h0_hist[:-1]
                h[0] = h0_new
        if pend is not None:
            emit_scores(*pend)
            emit_stag_copy(pend[1])

    # ---- post phase: window softmax + output assembly ----
    with contextlib.ExitStack() as ctx:
        fp = ctx.enter_context(tc.tile_pool(name="fields", bufs=1))
        tp = ctx.enter_context(tc.tile_pool(name="ptmp", bufs=2))
        stag3 = stag[:].rearrange("b (t q) -> b t q", q=8)
        osb = []
        for s in range(2):
            fld = []
            for q in range(4):
                if s == 1 and q == 3:
                    fld.append(None)  # stream-1 g field is never read
                    continue
                f = fp.tile([128, L], FP, tag=f"f{s}{q}")
                src = stag3[:, :, s * 4 + q:s * 4 + q + 1].rearrange(
                    "b t o -> b (t o)")
                nc.vector.tensor_copy(f[:], src)
                fld.append(f)
            sp, qf, pf, gf = fld

            def v(x, j):
                return x[:, j:j + WIN]

            m1 = tp.tile([128, WIN], FP, tag="m1")
            nc.vector.tensor_max(m1[:], v(sp, 0), v(sp, 1))
            m2 = tp.tile([128, WIN], FP, tag="m2")
            nc.vector.tensor_max(m2[:], v(sp, 2), v(sp, 3))
            m3 = tp.tile([128, WIN], FP, tag="m3")
            nc.vector.tensor_max(m3[:], m1[:], m2[:])
            mx = tp.tile([128, WIN], FP, tag="mx")
            nc.vector.tensor_max(mx[:], m3[:], v(sp, 4))
            es = []
            for j in range(ATTN):
                d = tp.tile([128, WIN], FP, tag=f"d{j}")
                nc.vector.tensor_sub(d[:], v(sp, j), mx[:])
                e = tp.tile([128, WIN], FP, tag=f"e{j}")
                nc.scalar.activation(e[:], d[:], AF.Exp)
                es.append(e)
            d01 = tp.tile([128, WIN], FP, tag="d01")
            nc.vector.tensor_add(d01[:], es[0][:], es[1][:])
            d23 = tp.tile([128, WIN], FP, tag="d23")
            nc.vector.tensor_add(d23[:], es[2][:], es[3][:])
            d03 = tp.tile([128, WIN], FP, tag="d03")
            nc.vector.tensor_add(d03[:], d01[:], d23[:])
            den = tp.tile([128, WIN], FP, tag="den")
            nc.vector.tensor_add(den[:], d03[:], es[4][:])
            # numerator: sum_j e_j * q(t-4+j)
            nums = []
            for j in range(ATTN):
                nmj = tp.tile([128, WIN], FP, tag=f"nm{j}")
                nc.gpsimd.tensor_mul(nmj[:], es[j][:], v(qf, j))
                nums.append(nmj)
            n01 = tp.tile([128, WIN], FP, tag="n01")
            nc.gpsimd.tensor_add(n01[:], nums[0][:], nums[1][:])
            n23 = tp.tile([128, WIN], FP, tag="n23")
            nc.gpsimd.tensor_add(n23[:], nums[2][:], nums[3][:])
            n03 = tp.tile([128, WIN], FP, tag="n03")
            nc.vector.tensor_add(n03[:], n01[:], n23[:])
            num = tp.tile([128, WIN], FP, tag="num")
            nc.vector.tensor_add(num[:], n03[:], nums[4][:])
            rec = tp.tile([128, WIN], FP, tag="rec")
            nc.vector.reciprocal(rec[:], den[:])
            att = tp.tile([128, WIN], FP, tag="att")
            nc.vector.tensor_mul(att[:], num[:], rec[:])
            ot = fp.tile([128, L], FP, tag=f"o{s}")
            # o = att + p + C_s  (C_s per-stream additive constant from blob)
            cs = aux[:, CONST_OFF + s:CONST_OFF + s + 1]
            nc.vector.scalar_tensor_tensor(
                ot[:, ATTN:L], att[:], cs, v(pf, ATTN), AL.add, AL.add)
            if s == 0:
                # passthrough region t<ATTN uses g = w_o_h . r1 (no b_c term)
                nc.vector.tensor_scalar_add(
                    ot[:, 0:ATTN], gf[:, 0:ATTN], 0.0)
            osb.append(ot)

        dec = fp.tile([128, L], FP, tag="dec")
        nc.vector.tensor_add(dec[:, 0:L - DLY], osb[0][:, 0:L - DLY],
                             osb[1][:, DLY:L])
        for i in range(DLY):
            c = L - DLY + i
            nc.vector.tensor_add(dec[:, c:c + 1], osb[0][:, c:c + 1],
                                 osb[1][:, L - 1:L])
        # u_b1 correction for passthrough cols is baked on host into g field
        sig = fp.tile([128, L], FP, tag="sig")
        nc.scalar.activation(sig[:], dec[:], AF.Sigmoid)
        nc.sync.dma_start(out_ap[:], sig[:])
        if stag_out is not None:
            nc.sync.dma_start(stag_out, stag[:])
    outer.close()


def _build(nc_count=NCORES):
    key = nc_count
    if key in _BUILT:
        return _BUILT[key]
    nc = bacc.Bacc("TRN2", target_bir_lowering=False, debug=False,
                   num_devices=nc_count)
    recv = nc.dram_tensor("recv", [NIN, L, BL], F16, kind="ExternalInput").ap()
    wb16 = nc.dram_tensor("wblob16", [128, NW16], F16,
                          kind="ExternalInput").ap()
    wb32 = nc.dram_tensor("wblob32", [128, NAUX], FP,
                          kind="ExternalInput").ap()
    onesd = nc.dram_tensor("onesd", [1, XCH * 128], F16,
                           kind="ExternalInput").ap()
    out_ap = nc.dram_tensor("out", [BL, L], FP, kind="ExternalOutput").ap()
    stag_out = None
    if os.environ.get("BASS_GRU_DBG"):
        stag_out = nc.dram_tensor("stag_out", [128, L * 8], FP,
                                  kind="ExternalOutput").ap()
    with tile.TileContext(nc) as tc:
        _emit(tc, recv, wb16, wb32, onesd, out_ap, stag_out)
    nc.compile()
    _BUILT[key] = nc
    return nc


def _prep_weights(inp):
    """Host-side packing of all weights into the fp16 + fp32 blobs."""
    wb = np.zeros((128, NW16), np.float32)

    def neg_z(m):  # m: [384, ...]; negate z gate rows
        m = m.copy()
        m[128:256] = -m[128:256]
        return m

    for s in range(2):
        sn = s + 1
        for l in range(2):
            whh = neg_z(np.asarray(inp[f"w_hh{sn}_l{l}"], np.float32))
            wb[:, HH_OFF + _cell(s, l) * 384:HH_OFF + _cell(s, l) * 384 + 384] = \
                whh.T  # [128, 384]
        # layer 1 plain lhsT
        wih1 = neg_z(np.asarray(inp[f"w_ih{sn}_l1"], np.float32))
        wb[:, IH1_OFF + s * 384:IH1_OFF + s * 384 + 384] = wih1.T
        # layer 0 augmented [3, 384]
        wih0 = neg_z(np.asarray(inp[f"w_ih{sn}_l0"], np.float32))
        b_ih0 = np.asarray(inp[f"b_ih{sn}_l0"], np.float32)
        b_hh0 = np.asarray(inp[f"b_hh{sn}_l0"], np.float32)
        aug = np.zeros((3, 384), np.float32)
        aug[0:2] = wih0.T
        aug[2, 0:128] = b_ih0[0:128] + b_hh0[0:128]
        aug[2, 128:256] = -(b_ih0[128:256] + b_hh0[128:256])
        aug[2, 256:384] = b_ih0[256:384]
        wb[:, IH0_OFF + s * 384:IH0_OFF + s * 384 + 384] = 0.0
        wb[0:3, IH0_OFF + s * 384:IH0_OFF + s * 384 + 384] = aug

    # score vectors
    w_a = np.asarray(inp["w_a"], np.float32)
    w_c = np.asarray(inp["w_c"], np.float32)
    b_c = np.asarray(inp["b_c"], np.float32)
    w_o = np.asarray(inp["w_o"], np.float32)
    b_o = float(np.asarray(inp["b_o"], np.float32)[0])
    consts = {}
    for s in range(2):
        wo_s = w_o[0, s * 128:(s + 1) * 128]
        u_c = wo_s @ w_c[:, 0:128]
        u_r = wo_s @ w_c[:, 128:256]
        u_b = float(wo_s @ b_c)
        sc = np.stack([w_a[0, 128:256], u_c, u_r, wo_s], axis=1)  # [128, 4]
        wb[:, SC_OFF + s * 4:SC_OFF + s * 4 + 4] = sc
        consts[f"u_b{s}"] = u_b
    consts["b_o"] = b_o

    # bias rows for the indicator matmuls:
    # slot 0 r-l1 [2,128], 1 z-l1 negated [2,128], 2 hn-l0 [2,128],
    # slot 3 nh/ni-l1 [4,128] (rows: hn_s0, hn_s1, in_s0, in_s1)
    for s in range(2):
        sn = s + 1
        b_ih1 = np.asarray(inp[f"b_ih{sn}_l1"], np.float32)
        b_hh1 = np.asarray(inp[f"b_hh{sn}_l1"], np.float32)
        b_hh0 = np.asarray(inp[f"b_hh{sn}_l0"], np.float32)
        wb[s, BR_OFF + 0 * 128:BR_OFF + 1 * 128] = b_ih1[0:128] + b_hh1[0:128]
        wb[s, BR_OFF + 1 * 128:BR_OFF + 2 * 128] = \
            -(b_ih1[128:256] + b_hh1[128:256])
        wb[s, BR_OFF + 2 * 128:BR_OFF + 3 * 128] = b_hh0[256:384]
        wb[s, BR_OFF + 3 * 128:BR_OFF + 4 * 128] = b_hh1[256:384]
        wb[s, BR_OFF + 4 * 128:BR_OFF + 5 * 128] = b_ih1[256:384]

    # indicator rhs IND4 [4, 512]: row k is 1.0 on cols [k*128,(k+1)*128)
    for k in range(4):
        wb[k, IND_OFF + k * 128:IND_OFF + (k + 1) * 128] = 1.0

    aux = np.zeros((128, NAUX), np.float32)
    # per-stream additive constants, broadcast down the partition dim:
    # stream 0 carries u_b0; stream 1 carries u_b1 + b_o.
    aux[:, CONST_OFF + 0] = consts["u_b0"]
    aux[:, CONST_OFF + 1] = consts["u_b1"] + b_o
    return wb.astype(np.float16), aux, consts


def kernel(**inputs):
    recv = np.ascontiguousarray(np.asarray(inputs["received"], np.float32))
    assert recv.shape == (B, L, NIN)
    assert int(inputs.get("attn_num", ATTN)) == ATTN
    assert int(inputs.get("d_delay", DLY)) == DLY
    wb16, wb32, _ = _prep_weights(inputs)
    nc = _build()
    core_ids = list(range(NCORES))
    ones = np.ones((1, XCH * 128), np.float16)
    in_maps = []
    for c in range(NCORES):
        shard = recv[c * BL:(c + 1) * BL]  # [BL, L, 2]
        in_maps.append({
            "recv": np.ascontiguousarray(
                shard.transpose(2, 1, 0)).astype(np.float16),
            "wblob16": wb16,
            "wblob32": wb32,
            "onesd": ones,
        })
    kw = {}
    if os.environ.get("BASS_GRU_TRACE"):
        kw = dict(trace=True, tmpdir=os.environ.get("BASS_GRU_TRACE_DIR",
                                                    "/tmp/gru_trace"))
    res = run_bass_kernel_spmd(nc, in_maps, core_ids, **kw)
    if getattr(res, "exec_time_ns", None) is not None:
        print(f"HW exec time: {res.exec_time_ns} ns", flush=True)
    outs = [res.results[i]["out"] for i in range(NCORES)]
    if os.environ.get("BASS_GRU_DBG"):
        global DBG_STAG
        DBG_STAG = [res.results[i]["stag_out"] for i in range(NCORES)]
    dec = np.concatenate(outs, axis=0)  # [1024, 512] -- sigmoid(dec_pre)
    return dec[..., None].astype(np.float32)


# revision 17
# speedup vs baseline: 1.0094x; 1.0094x over previous
"""Trainium2 Bass kernel for the 2-stack GRU decoder with 5-wide sliding
window attention (nn_DEC_59880434041064).

Strategy: pure data parallel over batch (1024 -> 8 cores x 128).
Per-core layout keeps features on partitions and batch on the free dim so
the sequential GRU needs no transposes.  The attention + output projection
is algebraically collapsed on the host: only four per-(b,t) scalar fields
(window score s_p, q = u_c.r, p = u_r.r, g = w_o.r) are needed, computed by
an N=4 matmul against each fresh hidden state, so no [H,B,T] context GEMM
ever runs on device.

v3 perf rework (on top of the fp16 v2), 2317us -> 1867us:
 - every per-step tile is PER-LAYER (incl. the z-gate psum + sigmoid that
   v2 shared): the tile tracker works at whole-tile granularity, so any
   shared tile serializes one layer's recurrence chain behind the other.
 - the (z-1)*h blend term moves to the idle Pool/GPSIMD engine as two ops
   (tensor_scalar z-1, then mult by h; Pool's ISA has no STT), taking
   ~650ns/step off the DVE, the busiest engine.
 - matmuls are emitted x-side/bias-indicator first (they open each psum
   bank with its single start=True), recurrent hh matmuls last, so the
   in-order PE queue never parks on ready work.  NOTE start=True clears
   has_written for the whole 2KB psum zero region, hence exactly one per
   bank; later matmuls rely on per-element has_written.
 - score matmuls lag one step so their fin1 wait cannot block the next
   step's hh matmuls at the PE queue head; x chunks are prefetched.
 - score fields stay SBUF-resident (per-chunk ACT copy into a persistent
   staging tile); the old SBUF->DRAM->SBUF round trip cost ~240us of
   tail latency.
 - bigger SBUF pool rotations let the tile scheduler elide WAR waits.
"""

import os

import numpy as np

import concourse.bass as bass
import concourse.mybir as mybir
import concourse.tile as tile
from concourse import bacc
from concourse.bass_utils import run_bass_kernel_spmd

FP = mybir.dt.float32
F16 = mybir.dt.float16
AL = mybir.AluOpType
AF = mybir.ActivationFunctionType

B, L, H, NIN = 1024, 512, 128, 2
L = int(os.environ.get("BASS_GRU_L", L))  # debug-size override
NCORES = 8
BL = B // NCORES            # 128 samples per core
ATTN, DLY = 5, 10
WIN = L - ATTN              # 507 valid attention positions (t >= 5)
XCH = min(64, L)            # timesteps per x-chunk DMA
SCH = min(64, L)            # timesteps per score-psum chunk

# fp16 weight blob column offsets (blob is [128, NW16] fp16)
HH_OFF = 0                  # 4 cells x 384
IH1_OFF = HH_OFF + 4 * 384  # 2 streams x 384 (layer-1 input weights)
IH0_OFF = IH1_OFF + 2 * 384  # 2 streams x 384 (layer-0 aug weights, rows 0..2)
SC_OFF = IH0_OFF + 2 * 384  # score vectors: 2 streams x 4 cols
BR_OFF = SC_OFF + 8         # bias rows for indicator MMs (see _prep_weights)
IND_OFF = BR_OFF + 5 * 128  # indicator rhs IND4 [4, 512]
NW16 = IND_OFF + 512

# fp32 aux blob (tiny): per-stream additive consts for the post phase
CONST_OFF = 0
NAUX = 2

_BUILT = {}


def _cell(s, l):
    return s * 2 + l


def _emit(tc, recv, wb16, wb32, onesd, out_ap, stag_out=None):
    nc = tc.nc
    import contextlib

    outer = contextlib.ExitStack()
    const = outer.enter_context(tc.tile_pool(name="const", bufs=1))
    wsb = const.tile([128, NW16], F16)
    nc.sync.dma_start(wsb[:], wb16[:])
    aux = const.tile([128, NAUX], FP)
    nc.sync.dma_start(aux[:], wb32[:])
    # persistent score staging: [b, t*8] fp16, written once per SCH chunk
    stag = const.tile([128, L * 8], FP)

    with contextlib.ExitStack() as ctx:
        xpool = ctx.enter_context(tc.tile_pool(name="xch", bufs=2))
        hpool = ctx.enter_context(tc.tile_pool(name="h", bufs=8))
        pw = ctx.enter_context(tc.tile_pool(name="pw", bufs=8))
        pg = [
            ctx.enter_context(tc.tile_pool(name=f"pg{l}", bufs=1, space="PSUM"))
            for l in range(2)
        ]
        psc = ctx.enter_context(tc.tile_pool(name="psc", bufs=2, space="PSUM"))

        def whh(s, l):  # [128, 384] lhsT
            o = HH_OFF + _cell(s, l) * 384
            return wsb[:, o:o + 384]

        def wih1(s):
            o = IH1_OFF + s * 384
            return wsb[:, o:o + 384]

        def wih0(s):  # [3, 384] augmented lhsT
            o = IH0_OFF + s * 384
            return wsb[0:3, o:o + 384]

        def scw(s):  # [128, 4]
            o = SC_OFF + s * 4
            return wsb[:, o:o + 4]

        # bias rows (lhsT for indicator matmuls); [2,128] per-stream groups:
        # slot 0 r-l1, 1 z-l1 (negated), 2 hn-l0, 3 hn-l1, 4 in-l1
        def brow(slot, k):
            o = BR_OFF + slot * 128
            return wsb[0:k, o:o + 128]

        def ind(k, n):  # [k, n] block indicator rhs
            return wsb[0:k, IND_OFF:IND_OFF + n]

        # initial hidden state (per layer, both streams concatenated)
        h = []
        for l in range(2):
            t0 = hpool.tile([128, 256], F16, tag=f"h{l}")
            nc.vector.memset(t0[:], 0.0)
            h.append(t0)

        def gru_stages(l, h_in, mm_rhs):
            """Build one fused (both-streams) GRU step for layer l as a dict
            of emit-stage callbacks, so the caller can interleave the two
            layers' independent chains stage-by-stage.  Every tile here is
            per-layer: sharing a tile across layers makes the tile tracker
            serialize one layer's chain behind the other (whole-tile
            granularity).

            mm_rhs: per-stream extra-input rhs ([3,128] aug x for l0,
            [128,128] r_l0 slice for l1).  Returns (stages, h')."""
            rp = pg[l].tile([128, 256], FP, tag=f"rp{l}", name=f"rp{l}")
            zp = pg[l].tile([128, 256], FP, tag=f"zp{l}", name=f"zp{l}")
            np_ = pg[l].tile([128, 512], FP, tag=f"np{l}", name=f"np{l}")
            nh = np_[:, 0:256]
            ni = np_[:, 256:512]
            rsb = pw.tile([128, 256], F16, tag=f"rsb{l}", name=f"rsb{l}")
            zsb = pw.tile([128, 256], F16, tag=f"zsb{l}", name=f"zsb{l}")
            tt = pw.tile([128, 256], F16, tag=f"tt{l}", name=f"tt{l}")
            uu = pw.tile([128, 256], F16, tag=f"uu{l}", name=f"uu{l}")
            nn_ = pw.tile([128, 256], F16, tag=f"nn{l}", name=f"nn{l}")
            zm = pw.tile([128, 256], F16, tag=f"zm{l}", name=f"zm{l}")
            hm = pw.tile([128, 256], F16, tag=f"hm{l}", name=f"hm{l}")
            zn = pw.tile([128, 256], F16, tag=f"zn{l}", name=f"zn{l}")
            hn_t = hpool.tile([128, 256], F16, tag=f"h{l}", name=f"hn{l}")

            def mm_early():
                # everything independent of this layer's fresh h: bias
                # indicators + x-side (l0) / lagged-rl0 (l1) input matmuls.
                # CRITICAL: start=True clears has_written for the whole 2KB
                # psum zero region (= bank), so each bank gets exactly ONE
                # start=True matmul, executed first; later matmuls rely on
                # per-element has_written (virgin columns are overwritten,
                # already-written ones accumulate).
                if l == 0:
                    nc.tensor.matmul(nh[:, 0:256], brow(2, 2), ind(2, 256),
                                     start=True, stop=False)
                    for s in range(2):
                        xr = mm_rhs[s]
                        wih = wih0(s)
                        c = s * 128
                        nc.tensor.matmul(rp[:, c:c + 128], wih[:, 0:128], xr,
                                         start=(s == 0), stop=False)
                        nc.tensor.matmul(zp[:, c:c + 128], wih[:, 128:256],
                                         xr, start=(s == 0), stop=False)
                        nc.tensor.matmul(ni[:, c:c + 128], wih[:, 256:384],
                                         xr, start=False, stop=False)
                else:
                    nc.tensor.matmul(rp[:, 0:256], brow(0, 2), ind(2, 256),
                                     start=True, stop=False)
                    nc.tensor.matmul(zp[:, 0:256], brow(1, 2), ind(2, 256),
                                     start=True, stop=False)
                    nc.tensor.matmul(nh[:, 0:256], brow(3, 2),
                                     ind(2, 256), start=True, stop=False)
                    nc.tensor.matmul(ni[:, 0:256], brow(4, 2),
                                     ind(2, 256), start=False, stop=False)
                    for s in range(2):
                        xr = mm_rhs[s]
                        wih = wih1(s)
                        c = s * 128
                        nc.tensor.matmul(rp[:, c:c + 128], wih[:, 0:128], xr,
                                         start=False, stop=False)
                        nc.tensor.matmul(zp[:, c:c + 128], wih[:, 128:256],
                                         xr, start=False, stop=False)
                        nc.tensor.matmul(ni[:, c:c + 128], wih[:, 256:384],
                                         xr, start=False, stop=False)

            def mm_hh(goff, dst):
                for s in range(2):
                    hs = h_in[:, s * 128:(s + 1) * 128]
                    c = s * 128
                    nc.tensor.matmul(dst[:, c:c + 128],
                                     whh(s, l)[:, goff:goff + 128], hs,
                                     start=False, stop=(s == 1))

            stages = {
                "early": mm_early,
                "hh_r": lambda: mm_hh(0, rp),
                "hh_z": lambda: mm_hh(128, zp),
                "hh_n": lambda: mm_hh(256, nh),
                "sr": lambda: nc.scalar.activation(rsb[:], rp[:], AF.Sigmoid),
                "sz": lambda: nc.scalar.activation(zsb[:], zp[:], AF.Sigmoid),
                "tt": lambda: nc.vector.tensor_mul(tt[:], nh[:], rsb[:]),
                "uu": lambda: nc.vector.tensor_add(uu[:], ni[:], tt[:]),
                "tanh": lambda: nc.scalar.activation(nn_[:], uu[:], AF.Tanh),
                # hm = (z'-1)*h on the otherwise-idle Pool engine; its
                # ISA has no scalar_tensor_tensor, so two legal ops
                "zm": lambda: nc.gpsimd.tensor_scalar_add(zm[:], zsb[:], -1.0),
                "hm": lambda: nc.gpsimd.tensor_mul(hm[:], zm[:], h_in[:]),
                "zn": lambda: nc.vector.tensor_mul(zn[:], zsb[:], nn_[:]),
                "fin": lambda: nc.vector.tensor_sub(hn_t[:], zn[:], hm[:]),
            }
            return stages, hn_t

        scp = None

        def emit_scores(h1_t, u):
            nonlocal scp
            if u % SCH == 0:
                scp = psc.tile([128, SCH * 8], FP, tag="scp", name=f"scp{u}")
            so = (u % SCH) * 8
            for s in range(2):
                nc.tensor.matmul(scp[:, so + s * 4:so + s * 4 + 4],
                                 h1_t[:, s * 128:(s + 1) * 128], scw(s),
                                 start=True, stop=True)

        def emit_stag_copy(u):
            # park the finished chunk's fields in the staging tile
            # (ACT copy: reads PSUM cheaply, DVE stays free)
            c0 = (u + 1 - SCH) * 8
            nc.scalar.copy(stag[:, c0:c0 + SCH * 8], scp[:])

        def load_chunk(c):
            xt = xpool.tile([3, XCH * 128], F16, tag="xch", name=f"xch{c}")
            src = recv[:, c * XCH:(c + 1) * XCH, :]
            dst = xt[0:2, :].rearrange("i (t b) -> i t b", b=128)
            nc.sync.dma_start(dst, src)
            nc.sync.dma_start(xt[2:3, :], onesd[:, 0:XCH * 128])
            return xt

        # wavefront: iteration tau emits l0(tau) and, LAG steps behind, the
        # l1 pair for tau-LAG -- the skew keeps the two recurrence chains
        # decoupled in every engine's in-order queue.  Scores for h1(u) are
        # emitted one iteration late so their fin1 wait never parks the PE
        # queue ahead of the next step's hh matmuls.
        LAG = 2
        h0_hist = [None] * LAG  # h0_hist[k] = h0 output of step tau-1-k
        xcur = load_chunk(0)
        xnext = None
        pend = None  # (h1 tile, u) for the delayed score matmuls
        for tau in range(L + LAG):
            st0 = st1 = None
            if tau < L:
                t = tau
                if t % XCH == 0 and t > 0:
                    xcur = xnext
                toff = t % XCH
                x_t = xcur[:, toff * 128:(toff + 1) * 128]
            if tau < L:
                st0, h0_new = gru_stages(0, h[0], [x_t, x_t])
            if tau >= LAG:
                u = tau - LAG
                # h0_hist stops rotating once tau >= L; account for that in
                # the drain iterations so the last l1 steps read fresh h0
                h0_u = h0_hist[LAG - 1 - max(0, tau - L)]
                rl0 = [h0_u[:, 0:128], h0_u[:, 128:256]]
                st1, h1_new = gru_stages(1, h[1], rl0)

            def run(st, k):
                if st is not None:
                    st[k]()

            # PE: no-fresh-h work first, then hh groups in dependency-
            # resolution order (fin0 lands before fin1); the delayed score
            # matmuls slot after hh_r0.
            run(st0, "early")
            run(st1, "early")
            run(st0, "hh_r")
            if pend is not None:
                emit_scores(*pend)
            run(st1, "hh_r")
            run(st0, "hh_n")
            run(st1, "hh_n")
            run(st0, "hh_z")
            run(st1, "hh_z")
            # ACT: each layer's z sigmoid right after its r sigmoid so
            # the blend helper (hm) unblocks early
            run(st0, "sr")
            run(st0, "sz")
            run(st1, "sr")
            run(st1, "sz")
            # Pool: blend helpers, each unblocked by its own z sigmoid
            run(st0, "zm")
            run(st0, "hm")
            run(st1, "zm")
            run(st1, "hm")
            # DVE: n-path pairs back-to-back, l0 first
            run(st0, "tt")
            run(st0, "uu")
            run(st1, "tt")
            run(st1, "uu")
            run(st0, "tanh")
            run(st1, "tanh")
            run(st0, "zn")
            run(st0, "fin")
            run(st1, "zn")
            run(st1, "fin")
            if pend is not None and (pend[1] + 1) % SCH == 0:
                emit_stag_copy(pend[1])
            pend = None
            # prefetch the next x chunk well before it is needed
            if tau < L and tau % XCH == 8 and t + XCH < L:
                xnext = load_chunk(t // XCH + 1)
            if tau >= LAG:
                pend = (h1_new, tau - LAG)
                h[1] = h1_new
            if tau < L:
                h0_hist = [h0_new] + h0_hist[:-1]
                h[0] = h0_new
        if pend is not None:
            emit_scores(*pend)
            emit_stag_copy(pend[1])

    # ---- post phase: window softmax + output assembly ----
    with contextlib.ExitStack() as ctx:
        fp = ctx.enter_context(tc.tile_pool(name="fields", bufs=1))
        tp = ctx.enter_context(tc.tile_pool(name="ptmp", bufs=2))
        stag3 = stag[:].rearrange("b (t q) -> b t q", q=8)
        osb = []
        for s in range(2):
            fld = []
            for q in range(4):
                if s == 1 and q == 3:
                    fld.append(None)  # stream-1 g field is never read
                    continue
                f = fp.tile([128, L], FP, tag=f"f{s}{q}")
                src = stag3[:, :, s * 4 + q:s * 4 + q + 1].rearrange(
                    "b t o -> b (t o)")
                nc.vector.tensor_copy(f[:], src)
                fld.append(f)
            sp, qf, pf, gf = fld

            def v(x, j):
                return x[:, j:j + WIN]

            # every window term e_j[t] = exp(sp[t-4+j]) (and e_j*q_j) is a
            # shifted view of ONE full-length exp / product: the whole
            # 5-wide softmax is 2 big ops + two 4-op add trees.  |sp| <= ~10
            # so unnormalized fp32 exp is safe (softmax is shift-invariant).
            ef = fp.tile([128, L], FP, tag=f"ef{s}")
            nc.scalar.activation(ef[:], sp[:], AF.Exp)
            eq = fp.tile([128, L], FP, tag=f"eq{s}")
            nc.gpsimd.tensor_mul(eq[:], ef[:], qf[:])
            d01 = tp.tile([128, WIN], FP, tag="d01")
            nc.vector.tensor_add(d01[:], v(ef, 0), v(ef, 1))
            d23 = tp.tile([128, WIN], FP, tag="d23")
            nc.vector.tensor_add(d23[:], v(ef, 2), v(ef, 3))
            d03 = tp.tile([128, WIN], FP, tag="d03")
            nc.vector.tensor_add(d03[:], d01[:], d23[:])
            den = tp.tile([128, WIN], FP, tag="den")
            nc.vector.tensor_add(den[:], d03[:], v(ef, 4))
            n01 = tp.tile([128, WIN], FP, tag="n01")
            nc.gpsimd.tensor_add(n01[:], v(eq, 0), v(eq, 1))
            n23 = tp.tile([128, WIN], FP, tag="n23")
            nc.gpsimd.tensor_add(n23[:], v(eq, 2), v(eq, 3))
            n03 = tp.tile([128, WIN], FP, tag="n03")
            nc.gpsimd.tensor_add(n03[:], n01[:], n23[:])
            num = tp.tile([128, WIN], FP, tag="num")
            nc.vector.tensor_add(num[:], n03[:], v(eq, 4))
            rec = tp.tile([128, WIN], FP, tag="rec")
            nc.vector.reciprocal(rec[:], den[:])
            att = tp.tile([128, WIN], FP, tag="att")
            nc.vector.tensor_mul(att[:], num[:], rec[:])
            ot = fp.tile([128, L], FP, tag=f"o{s}")
            # o = att + p + C_s  (C_s per-stream additive constant from blob)
            cs = aux[:, CONST_OFF + s:CONST_OFF + s + 1]
            nc.vector.scalar_tensor_tensor(
                ot[:, ATTN:L], att[:], cs, v(pf, ATTN), AL.add, AL.add)
            if s == 0:
                # passthrough region t<ATTN uses g = w_o_h . r1 (no b_c term)
                nc.vector.tensor_scalar_add(
                    ot[:, 0:ATTN], gf[:, 0:ATTN], 0.0)
            osb.append(ot)

        dec = fp.tile([128, L], FP, tag="dec")
        nc.vector.tensor_add(dec[:, 0:L - DLY], osb[0][:, 0:L - DLY],
                             osb[1][:, DLY:L])
        for i in range(DLY):
            c = L - DLY + i
            nc.vector.tensor_add(dec[:, c:c + 1], osb[0][:, c:c + 1],
                                 osb[1][:, L - 1:L])
        # u_b1 correction for passthrough cols is baked on host into g field
        sig = fp.tile([128, L], FP, tag="sig")
        nc.scalar.activation(sig[:], dec[:], AF.Sigmoid)
        nc.sync.dma_start(out_ap[:], sig[:])
        if stag_out is not None:
            nc.sync.dma_start(stag_out, stag[:])
    outer.close()


def _build(nc_count=NCORES):
    key = nc_count
    if key in _BUILT:
        return _BUILT[key]
    nc = bacc.Bacc("TRN2", target_bir_lowering=False, debug=False,
                   num_devices=nc_count)
    recv = nc.dram_tensor("recv", [NIN, L, BL], F16, kind="ExternalInput").ap()
    wb16 = nc.dram_tensor("wblob16", [128, NW16], F16,
                          kind="ExternalInput").ap()
    wb32 = nc.dram_tensor("wblob32", [128, NAUX], FP,
                          kind="ExternalInput").ap()
    onesd = nc.dram_tensor("onesd", [1, XCH * 128], F16,
                           kind="ExternalInput").ap()
    out_ap = nc.dram_tensor("out", [BL, L], FP, kind="ExternalOutput").ap()
    stag_out = None
    if os.environ.get("BASS_GRU_DBG"):
        stag_out = nc.dram_tensor("stag_out", [128, L * 8], FP,
                                  kind="ExternalOutput").ap()
    with tile.TileContext(nc) as tc:
        _emit(tc, recv, wb16, wb32, onesd, out_ap, stag_out)
    nc.compile()
    _BUILT[key] = nc
    return nc


def _prep_weights(inp):
    """Host-side packing of all weights into the fp16 + fp32 blobs."""
    wb = np.zeros((128, NW16), np.float32)

    def neg_z(m):  # m: [384, ...]; negate z gate rows
        m = m.copy()
        m[128:256] = -m[128:256]
        return m

    for s in range(2):
        sn = s + 1
        for l in range(2):
            whh = neg_z(np.asarray(inp[f"w_hh{sn}_l{l}"], np.float32))
            wb[:, HH_OFF + _cell(s, l) * 384:HH_OFF + _cell(s, l) * 384 + 384] = \
                whh.T  # [128, 384]
        # layer 1 plain lhsT
        wih1 = neg_z(np.asarray(inp[f"w_ih{sn}_l1"], np.float32))
        wb[:, IH1_OFF + s * 384:IH1_OFF + s * 384 + 384] = wih1.T
        # layer 0 augmented [3, 384]
        wih0 = neg_z(np.asarray(inp[f"w_ih{sn}_l0"], np.float32))
        b_ih0 = np.asarray(inp[f"b_ih{sn}_l0"], np.float32)
        b_hh0 = np.asarray(inp[f"b_hh{sn}_l0"], np.float32)
        aug = np.zeros((3, 384), np.float32)
        aug[0:2] = wih0.T
        aug[2, 0:128] = b_ih0[0:128] + b_hh0[0:128]
        aug[2, 128:256] = -(b_ih0[128:256] + b_hh0[128:256])
        aug[2, 256:384] = b_ih0[256:384]
        wb[:, IH0_OFF + s * 384:IH0_OFF + s * 384 + 384] = 0.0
        wb[0:3, IH0_OFF + s * 384:IH0_OFF + s * 384 + 384] = aug

    # score vectors
    w_a = np.asarray(inp["w_a"], np.float32)
    w_c = np.asarray(inp["w_c"], np.float32)
    b_c = np.asarray(inp["b_c"], np.float32)
    w_o = np.asarray(inp["w_o"], np.float32)
    b_o = float(np.asarray(inp["b_o"], np.float32)[0])
    consts = {}
    for s in range(2):
        wo_s = w_o[0, s * 128:(s + 1) * 128]
        u_c = wo_s @ w_c[:, 0:128]
        u_r = wo_s @ w_c[:, 128:256]
        u_b = float(wo_s @ b_c)
        sc = np.stack([w_a[0, 128:256], u_c, u_r, wo_s], axis=1)  # [128, 4]
        wb[:, SC_OFF + s * 4:SC_OFF + s * 4 + 4] = sc
        consts[f"u_b{s}"] = u_b
    consts["b_o"] = b_o

    # bias rows for the indicator matmuls:
    # slot 0 r-l1 [2,128], 1 z-l1 negated [2,128], 2 hn-l0 [2,128],
    # slot 3 nh/ni-l1 [4,128] (rows: hn_s0, hn_s1, in_s0, in_s1)
    for s in range(2):
        sn = s + 1
        b_ih1 = np.asarray(inp[f"b_ih{sn}_l1"], np.float32)
        b_hh1 = np.asarray(inp[f"b_hh{sn}_l1"], np.float32)
        b_hh0 = np.asarray(inp[f"b_hh{sn}_l0"], np.float32)
        wb[s, BR_OFF + 0 * 128:BR_OFF + 1 * 128] = b_ih1[0:128] + b_hh1[0:128]
        wb[s, BR_OFF + 1 * 128:BR_OFF + 2 * 128] = \
            -(b_ih1[128:256] + b_hh1[128:256])
        wb[s, BR_OFF + 2 * 128:BR_OFF + 3 * 128] = b_hh0[256:384]
        wb[s, BR_OFF + 3 * 128:BR_OFF + 4 * 128] = b_hh1[256:384]
        wb[s, BR_OFF + 4 * 128:BR_OFF + 5 * 128] = b_ih1[256:384]

    # indicator rhs IND4 [4, 512]: row k is 1.0 on cols [k*128,(k+1)*128)
    for k in range(4):
        wb[k, IND_OFF + k * 128:IND_OFF + (k + 1) * 128] = 1.0

    aux = np.zeros((128, NAUX), np.float32)
    # per-stream additive constants, broadcast down the partition dim:
    # stream 0 carries u_b0; stream 1 carries u_b1 + b_o.
    aux[:, CONST_OFF + 0] = consts["u_b0"]
    aux[:, CONST_OFF + 1] = consts["u_b1"] + b_o
    return wb.astype(np.float16), aux, consts


def kernel(**inputs):
    recv = np.ascontiguousarray(np.asarray(inputs["received"], np.float32))
    assert recv.shape == (B, L, NIN)
    assert int(inputs.get("attn_num", ATTN)) == ATTN
    assert int(inputs.get("d_delay", DLY)) == DLY
    wb16, wb32, _ = _prep_weights(inputs)
    nc = _build()
    core_ids = list(range(NCORES))
    ones = np.ones((1, XCH * 128), np.float16)
    in_maps = []
    for c in range(NCORES):
        shard = recv[c * BL:(c + 1) * BL]  # [BL, L, 2]
        in_maps.append({
            "recv": np.ascontiguousarray(
                shard.transpose(2, 1, 0)).astype(np.float16),
            "wblob16": wb16,
            "wblob32": wb32,
            "onesd": ones,
        })
    kw = {}
    if os.environ.get("BASS_GRU_TRACE"):
        kw = dict(trace=True, tmpdir=os.environ.get("BASS_GRU_TRACE_DIR",
                                                    "/tmp/gru_trace"))
    res = run_bass_kernel_spmd(nc, in_maps, core_ids, **kw)
    if getattr(res, "exec_time_ns", None) is not None:
        print(f"HW exec time: {res.exec_time_ns} ns", flush=True)
    outs = [res.results[i]["out"] for i in range(NCORES)]
    if os.environ.get("BASS_GRU_DBG"):
        global DBG_STAG
        DBG_STAG = [res.results[i]["stag_out"] for i in range(NCORES)]
    dec = np.concatenate(outs, axis=0)  # [1024, 512] -- sigmoid(dec_pre)
    return dec[..., None].astype(np.float32)
